# revision 19
# baseline (speedup 1.0000x reference)
"""Tensor-parallel causal attention (GQA, rotary) for Trainium2, 8 NeuronCores.

Problem: x[2,2048,2048] -> QKV proj -> rotary -> 32-head causal attention
(8 kv heads, head_dim 64) -> out @ wo, fp32 reference.

Sharding (batch x head-group): core c = 4*b + g owns batch b and q heads
[8g, 8g+8) with kv heads {2g, 2g+1}.  Each core computes its heads'
attention for its batch and a partial output projection
partial_c = attn_c @ wo[512g:512g+512]; the host sums 4 partials per batch.
This halves per-core x-read and OUT-write vs 8-way head-parallel.

Per-core pipeline (single TileContext; instruction-interleaved emission):
  A) Fused QKV projection from host-pre-transposed bf16 x (xT [D, T]) in 4
     token blocks x 2 column groups (cg = 4 q heads + 1 kv head, 384 W
     cols, pre-permuted so rotary even/odd pairs arrive deinterleaved).
     The QKV psums are evicted to bf16 SBUF by fast ACT copies (freeing
     the psum work slots in ~0.6us instead of holding them through the
     rotary chain); rotary runs all-bf16 on DVE at the 2x perf mode and
     writes bf16 QF pair tiles / KF (K replicated to rows 64:128 for
     2-head row-packing); V.T is PE-transposed into bf16 (V|1) tiles
     whose ones column makes the PV matmul emit softmax denominators
     (ones written once at startup via a strided memset).
  C) Per q-block j (512 queries) and head pair pr: scores transposed
     S.T[k,q] = K.T-tile @ Q.T, two heads row-packed per PE pass; exp on
     ACT with 1/sqrt(64) folded into the activation scale; causal masking
     via ONE gpsimd affine_select per diagonal tile restricted to the
     128-column crossing block; fully-masked columns skipped at matmul
     granularity.  PV accumulates (V|1).T @ P.T into a [65,1024] psum
     (both heads of the pair); row 64 is the denominator.  Unnormalized PV
     is evicted to SBUF; denominator rows land on partitions 32*pr of a
     shared tile (32-aligned shifts), so one ACT Ln + one ACT
     Exp(scale=-1) computes 1/den for all 4 pairs per j (Ln and Exp share
     the natural_log_exp table set - no table thrash).  Ones-matmul
     broadcasts (PE) + DVE muls write normalized bf16 ATT tiles.
  D) Output projection per j: ATT stationary vs bf16 WO moving into
     [128,512] work psums; bf16 OUT partials DMA'd on both queues.

  PE-idle avoidance: exp (ACT) gates PV in phase C, so (i) the t-loop is
  software-pipelined (scores(t+1) is emitted before PV(t) so the in-order
  PE never waits on exp), and (ii) phase-A blocks n>=1 and phase-D output
  projections are emitted as small "filler" generator units popped
  between score tiles - the PE stays dense, keeping the HAM clock-gate
  warm (out-projections are deferred to the later, larger exp windows:
  op(0)->j2, op(1..2)->j3).  The single-shot NTFF span is the metric, so
  startup DMAs are emitted in first-use order and Exp/Ln are pinned to
  the shared natural_log_exp ACT table set via a compile-time patch.

PSUM (8 banks): sc [128,1024]f32 x2 = 4, pv [65,1024]f32 x1 = 2,
work [128,512]f32 x2 = 2 (time-shared by QKV accum / V-transpose /
out-proj / denominator broadcast).
"""
import numpy as np

B, S, D = 2, 2048, 2048
H, KV, HD = 32, 8, 64
NCORES = 8
NGRP = 4                    # head groups (tensor-parallel dimension)
HPG = H // NGRP             # 8 q heads per group
T = S                       # 2048 tokens per core (one batch)
DCH = D // 128              # 16 contraction chunks
NBLK = T // 512             # 4 token blocks of 512
KTILES = S // 128           # 16 k tiles
PAIRS = HPG // 2            # 4 row-packed head pairs per core

_CACHE = {}


def _build(reps=1, phases="acd", interleave=True):
    """reps>1 statically unrolls the whole pipeline for timing runs."""
    from collections import deque
    import concourse.bacc as bacc
    import concourse.mybir as mybir
    from concourse import tile
    from concourse.masks import make_identity

    F32 = mybir.dt.float32
    BF16 = mybir.dt.bfloat16
    EXP = mybir.ActivationFunctionType.Exp
    LN = mybir.ActivationFunctionType.Ln

    nc = bacc.Bacc()
    xT = nc.declare_dram_parameter("xT", [D, T], BF16, isOutput=False)
    W = nc.declare_dram_parameter("W", [D, 768], BF16, isOutput=False)
    WO = nc.declare_dram_parameter("WO", [512, D], BF16, isOutput=False)
    CS = nc.declare_dram_parameter("CS", [128, S], BF16, isOutput=False)
    SN = nc.declare_dram_parameter("SN", [128, S], BF16, isOutput=False)
    OUT = nc.declare_dram_parameter("OUT", [T, D], BF16, isOutput=True)

    with tile.TileContext(nc) as tc:
        with (
            tc.tile_pool(name="const", bufs=1) as cp,
            tc.tile_pool(name="xa", bufs=8) as xap,
            tc.tile_pool(name="tap", bufs=2) as tap,
            tc.tile_pool(name="ptp", bufs=6) as ptp,
            tc.tile_pool(name="aup", bufs=1) as aup,
            tc.tile_pool(name="dnp", bufs=1) as dnp,
            tc.tile_pool(name="odp", bufs=3) as odp,
            tc.tile_pool(name="ps", bufs=1, space="PSUM") as psp,
        ):
            QF = cp.tile([128, PAIRS * T], BF16)     # pair p at cols p*T
            KF = cp.tile([128, 2 * T], BF16)         # kv k at cols k*T; 0:64 + replica
            VH = cp.tile([128, 2 * KTILES * 65], BF16)
            ATT = cp.tile([128, PAIRS * T], BF16)    # pair f at cols f*T
            W_sb = cp.tile([128, DCH * 768], BF16)
            WO_sb = cp.tile([128, 4 * D], BF16)      # f chunk at cols f*D
            CS_sb = cp.tile([128, S], BF16)
            SN_sb = cp.tile([128, S], BF16)
            ident = cp.tile([128, 128], F32)
            identb = cp.tile([128, 128], BF16)
            ones128f = cp.tile([128, 64], F32)
            ones128 = cp.tile([128, 64], BF16)

            def emit_x_dmas(n):
                # 4 quarter tiles [128, 4 chunks * 512 toks] bf16 per block
                xr = xT.rearrange("(k p) t -> p k t", p=128)
                ncols = slice(n * 512, (n + 1) * 512)
                xhs = []
                for hh in range(4):
                    xh = xap.tile([128, 4 * 512], BF16, tag="xt",
                                  name=f"xt_{n}_{hh}")
                    nc.sync.dma_start(out=xh[:], in_=xr[:, hh * 4:(hh + 1) * 4, ncols])
                    xhs.append(xh)
                return xhs

            # emission order tracks first-use time.  The scalar (ACT) engine
            # issues NO DMAs: its queue must stay pure for exp.  sync carries
            # x (quarter 0 of block 0 split per chunk so matmul k=0 starts on
            # ~128KB instead of 512KB) interleaved with odd W chunks; gpsimd
            # carries even W chunks + CS/SN.
            xr0 = xT.rearrange("(k p) t -> p k t", p=128)
            xhs0 = [xap.tile([128, 4 * 512], BF16, tag="xt", name=f"xt_0_{hh}")
                    for hh in range(4)]
            for k in range(4):  # quarter 0, per-chunk split
                nc.sync.dma_start(out=xhs0[0][:, k * 512:(k + 1) * 512],
                                  in_=xr0[:, k:k + 1, 0:512])
                if k % 2 == 1:
                    nc.sync.dma_start(out=W_sb[:, k * 768:(k + 1) * 768],
                                      in_=W[k * 128:(k + 1) * 128, :])
            for hh in range(1, 4):
                nc.sync.dma_start(out=xhs0[hh][:],
                                  in_=xr0[:, hh * 4:(hh + 1) * 4, 0:512])
                for k in range(4 * hh + 1, 4 * hh + 4, 2):
                    nc.sync.dma_start(out=W_sb[:, k * 768:(k + 1) * 768],
                                      in_=W[k * 128:(k + 1) * 128, :])
            for k in range(0, 8, 2):
                nc.gpsimd.dma_start(out=W_sb[:, k * 768:(k + 1) * 768],
                                    in_=W[k * 128:(k + 1) * 128, :])
            nc.gpsimd.dma_start(out=CS_sb[:], in_=CS[:])
            for k in range(8, DCH, 2):
                nc.gpsimd.dma_start(out=W_sb[:, k * 768:(k + 1) * 768],
                                    in_=W[k * 128:(k + 1) * 128, :])
            nc.gpsimd.dma_start(out=SN_sb[:], in_=SN[:])
            xhs1 = emit_x_dmas(1)
            for f in range(4):
                nc.sync.dma_start(out=WO_sb[:, f * D:(f + 1) * D],
                                  in_=WO[f * 128:(f + 1) * 128, :])
            make_identity(nc, ident[:])
            nc.vector.tensor_copy(identb[:], ident[:])
            nc.vector.memset(ones128f[:], 1.0)
            nc.vector.tensor_copy(ones128[:], ones128f[:])
            # (V|1) ones columns never change - write them once
            nc.vector.memset(
                VH[:].rearrange("p (s c) -> p s c", c=65)[:, :, 64:65], 1.0)

            def _emit_body(rep):
              queue = deque()

              def pop(n=1):
                  for _ in range(n):
                      while queue:
                          try:
                              next(queue[0])
                              break
                          except StopIteration:
                              queue.popleft()

              def drain():
                  while queue:
                      pop()

              # -------- Phase A generator: one token block (512 toks, 2 cgs)
              def a_block_gen(n, xhs, cgs=(0, 1)):
                  ncols = slice(n * 512, (n + 1) * 512)
                  for cg in cgs:
                      wof = lambda k, m: k * 768 + cg * 384 + m * 128
                      ps_e = psp.tile([128, 512], F32, tag="work", bufs=2,
                                      name=f"pse_{rep}_{n}_{cg}")
                      ps_o = psp.tile([128, 512], F32, tag="work", bufs=2,
                                      name=f"pso_{rep}_{n}_{cg}")
                      for kg in range(4):
                          for k in range(4 * kg, 4 * kg + 4):
                              xt = xhs[k // 4][:, (k % 4) * 512:(k % 4 + 1) * 512]
                              nc.tensor.matmul(ps_e[:], W_sb[:, wof(k, 0):wof(k, 0) + 128],
                                               xt, start=(k == 0), stop=(k == DCH - 1))
                              nc.tensor.matmul(ps_o[:], W_sb[:, wof(k, 1):wof(k, 1) + 128],
                                               xt, start=(k == 0), stop=(k == DCH - 1))
                          yield
                      # evict the Q psums to bf16 SBUF on ACT (fast, frees the
                      # work slots immediately); rotary then runs all-bf16 on
                      # DVE at the 2x perf mode
                      se = tap.tile([128, 512], BF16, tag="se", bufs=3, name=f"se_{rep}_{n}_{cg}")
                      so = tap.tile([128, 512], BF16, tag="se", bufs=3, name=f"so_{rep}_{n}_{cg}")
                      nc.scalar.copy(se[:], ps_e[:])
                      nc.scalar.copy(so[:], ps_o[:])
                      t1 = tap.tile([128, 512], BF16, tag="t1", name=f"t1_{rep}_{n}_{cg}")
                      t2 = tap.tile([128, 512], BF16, tag="t2", name=f"t2_{rep}_{n}_{cg}")
                      nc.vector.tensor_mul(t1[:], se[:], CS_sb[:, ncols])
                      nc.vector.tensor_mul(t2[:], so[:], SN_sb[:, ncols])
                      for h in range(4):
                          p = 2 * cg + h // 2
                          base = p * T + n * 512
                          nc.vector.tensor_sub(
                              QF[64 * (h % 2):64 * (h % 2) + 32, base:base + 512],
                              t1[32 * h:32 * h + 32, :], t2[32 * h:32 * h + 32, :])
                      yield
                      t3 = tap.tile([128, 512], BF16, tag="t1", name=f"t3_{rep}_{n}_{cg}")
                      t4 = tap.tile([128, 512], BF16, tag="t2", name=f"t4_{rep}_{n}_{cg}")
                      nc.vector.tensor_mul(t3[:], se[:], SN_sb[:, ncols])
                      nc.vector.tensor_mul(t4[:], so[:], CS_sb[:, ncols])
                      for h in range(4):
                          p = 2 * cg + h // 2
                          base = p * T + n * 512
                          nc.vector.tensor_add(
                              QF[64 * (h % 2) + 32:64 * (h % 2) + 64, base:base + 512],
                              t3[32 * h:32 * h + 32, :], t4[32 * h:32 * h + 32, :])
                      yield
                      # K/V projection for this cg's kv head
                      ps_kv = psp.tile([128, 512], F32, tag="work", bufs=2,
                                       name=f"pskv_{rep}_{n}_{cg}")
                      for kg in range(4):
                          for k in range(4 * kg, 4 * kg + 4):
                              xt = xhs[k // 4][:, (k % 4) * 512:(k % 4 + 1) * 512]
                              nc.tensor.matmul(ps_kv[:], W_sb[:, wof(k, 2):wof(k, 2) + 128],
                                               xt, start=(k == 0), stop=(k == DCH - 1))
                          if kg % 2 == 1:
                              yield
                      # evict K/V psum to bf16 SBUF (kvb: rows 0:64 K, 64:128 V)
                      kvb = tap.tile([128, 512], BF16, tag="se", bufs=3, name=f"kvb_{rep}_{n}_{cg}")
                      nc.scalar.copy(kvb[:], ps_kv[:])
                      # rotary K: rows 0:32 even, 32:64 odd
                      kvcols = slice(cg * T + n * 512, cg * T + (n + 1) * 512)
                      tk1 = tap.tile([32, 512], BF16, tag="t1", name=f"tk1_{rep}_{n}_{cg}")
                      tk2 = tap.tile([32, 512], BF16, tag="t2", name=f"tk2_{rep}_{n}_{cg}")
                      nc.vector.tensor_mul(tk1[:], kvb[0:32, :], CS_sb[0:32, ncols])
                      nc.vector.tensor_mul(tk2[:], kvb[32:64, :], SN_sb[32:64, ncols])
                      nc.vector.tensor_sub(KF[0:32, kvcols], tk1[:], tk2[:])
                      tk3 = tap.tile([32, 512], BF16, tag="t1", name=f"tk3_{rep}_{n}_{cg}")
                      tk4 = tap.tile([32, 512], BF16, tag="t2", name=f"tk4_{rep}_{n}_{cg}")
                      nc.vector.tensor_mul(tk3[:], kvb[0:32, :], SN_sb[0:32, ncols])
                      nc.vector.tensor_mul(tk4[:], kvb[32:64, :], CS_sb[32:64, ncols])
                      nc.vector.tensor_add(KF[32:64, kvcols], tk3[:], tk4[:])
                      keng = nc.sync if n % 2 == 0 else nc.gpsimd
                      keng.dma_start(out=KF[64:128, kvcols], in_=KF[0:64, kvcols])
                      yield
                      # V.T: PE-transpose bf16 V rows into VH
                      for q in range(4):
                          tg = n * 4 + q
                          idx = (cg * KTILES + tg) * 65
                          vt_ps = psp.tile([128, 64], BF16, tag="work", bufs=2,
                                           name=f"vt_{rep}_{n}_{cg}_{q}")
                          nc.tensor.transpose(vt_ps[:],
                                              kvb[64:128, q * 128:(q + 1) * 128],
                                              identb[64:128, 64:128])
                          nc.scalar.copy(VH[:, idx:idx + 64], vt_ps[:])
                          if q % 2 == 1:
                              yield

              # -------- Phase D generator: out-projection for q-block j
              # Evictions avoid the scalar (ACT) engine: the op units run as
              # filler inside the exp-bound j2/j3 windows, where every ACT
              # cycle is critical.  nb0 -> vector, nb1 -> gpsimd.  The final
              # block's last unit evicts on ACT (idle by then) and splits its
              # OUT DMA across sync+gpsimd so the tail drains early.
              def outproj_gen(j, mqs=(0, 1, 2, 3), nb1_vec=False):
                  last_j = j == NBLK - 1
                  for mq in mqs:
                      mt = j * 4 + mq
                      for hf in range(2):
                          is_last = last_j and mq == 3
                          os_ = odp.tile([128, 1024], BF16, tag="od",
                                         name=f"od_{rep}_{mt}_{hf}")
                          for nb in range(2):
                              ps = psp.tile([128, 512], F32, tag="work", bufs=2,
                                            name=f"pd_{rep}_{mt}_{hf}_{nb}")
                              col = hf * 1024 + nb * 512
                              for f in range(4):
                                  nc.tensor.matmul(
                                      ps[:],
                                      ATT[:, f * T + mt * 128:f * T + (mt + 1) * 128],
                                      WO_sb[:, f * D + col:f * D + col + 512],
                                      start=(f == 0), stop=(f == 3))
                              dst = os_[:, nb * 512:(nb + 1) * 512]
                              if nb == 0 or nb1_vec:
                                  nc.vector.tensor_copy(dst, ps[:])
                              else:
                                  nc.scalar.copy(dst, ps[:])
                              if is_last:
                                  (nc.sync if nb == 0 else nc.gpsimd).dma_start(
                                      out=OUT[mt * 128:(mt + 1) * 128,
                                              hf * 1024 + nb * 512:
                                              hf * 1024 + (nb + 1) * 512],
                                      in_=dst)
                              yield
                          if not is_last:
                              nc.sync.dma_start(
                                  out=OUT[mt * 128:(mt + 1) * 128,
                                          hf * 1024:(hf + 1) * 1024],
                                  in_=os_[:])

              # -------- main schedule
              if rep == 0:
                  xcur = [xhs0, xhs1]
              else:
                  xcur = [emit_x_dmas(0), emit_x_dmas(1)]
              # drain cg0 of block 0, then let cg1 ride the filler queue so
              # prs(0) pairs 0/1 (which need only cg0) start ~8us earlier
              queue.append(a_block_gen(0, xcur[0], cgs=(0,)))
              drain()
              a0cg1 = a_block_gen(0, xcur[0], cgs=(1,)) if "c" in phases else None
              if a0cg1 is not None:
                  queue.append(a0cg1)

              if "c" not in phases:
                  return

              op_gens = {}
              for j in range(NBLK):
                  if j + 1 < NBLK:
                      queue.append(a_block_gen(j + 1, xcur[j + 1]))
                      pop(2)
                  if j + 2 < NBLK:
                      xcur.append(emit_x_dmas(j + 2))
                  # out-projections are deferred toward the later (bigger)
                  # ACT-bound windows: op(0) fills j=2, op(1) fills j=3's
                  # t-loop; op(2) is held back for j=3's normalization window
                  # (queued below) so the PE has filler while recip/bc run.
                  if j == 2 and 0 in op_gens:
                      queue.append(op_gens.pop(0))
                  if j == 3 and 1 in op_gens:
                      queue.append(op_gens.pop(1))
                  attu = []
                  denj = dnp.tile([128, 1024], F32, tag="den", name=f"den_{rep}_{j}")
                  for pr in range(PAIRS):
                      # op(2)'s first half rides the queue from the last
                      # pair's t-loop; the second half is reserved for the
                      # normalization window (queued below).
                      if j == 3 and pr == 3 and "2a" in op_gens:
                          queue.append(op_gens.pop("2a"))
                      if j == 0 and pr == 2 and a0cg1 is not None:
                          # pairs 2/3 consume cg1 outputs: cg1 must be fully
                          # EMITTED before their reads are traced (else the
                          # dep tracker sees reads-before-writes)
                          while a0cg1 in queue:
                              pop()
                      kv = pr // 2
                      nk = 4 * (j + 1)
                      pv = psp.tile([65, 1024], F32, tag="pv", name=f"pv_{rep}_{j}_{pr}")

                      def emit_pv(pt, qskip, t, nk=nk, pv=pv, kv=kv):
                          vcol = VH[:, (kv * KTILES + t) * 65:(kv * KTILES + t) * 65 + 65]
                          nc.tensor.matmul(pv[0:65, qskip:512], vcol,
                                           pt[:, qskip:512],
                                           start=(t == 0), stop=(t == nk - 1))
                          nc.tensor.matmul(pv[0:65, 512 + qskip:1024], vcol,
                                           pt[:, 512 + qskip:1024],
                                           start=(t == 0), stop=(t == nk - 1))

                      pending = []
                      for t in range(nk):
                          kc = slice(kv * T + t * 128, kv * T + (t + 1) * 128)
                          i = t - 4 * j
                          qskip = max(0, i * 128)
                          q0 = pr * T + j * 512
                          sc = psp.tile([128, 1024], F32, tag="sc", bufs=2,
                                        name=f"sc_{rep}_{j}_{pr}_{t}")
                          nc.tensor.matmul(sc[:, qskip:512], KF[0:64, kc],
                                           QF[0:64, q0 + qskip:q0 + 512],
                                           start=True, stop=True)
                          nc.tensor.matmul(sc[:, 512 + qskip:1024], KF[64:128, kc],
                                           QF[64:128, q0 + qskip:q0 + 512],
                                           start=True, stop=True)
                          pt = ptp.tile([128, 1024], BF16, tag="pt",
                                        name=f"pt_{rep}_{j}_{pr}_{t}")
                          if qskip == 0:
                              nc.scalar.activation(pt[:], sc[:], EXP, scale=0.125)
                          else:
                              pt3 = pt[:].rearrange("p (g q) -> p g q", g=2)[
                                  :, :, qskip:512]
                              sc3 = sc[:].rearrange("p (g q) -> p g q", g=2)[
                                  :, :, qskip:512]
                              nc.scalar.activation(pt3, sc3, EXP, scale=0.125)
                          if i >= 0:
                              # causal mask only on the 128-col crossing block
                              # of each head half: keep where qf_local >= kp
                              sel = pt[:].rearrange("p (g q) -> p g q", g=2)[
                                  :, :, qskip:qskip + 128]
                              nc.gpsimd.affine_select(
                                  out=sel, in_=sel,
                                  compare_op=mybir.AluOpType.is_ge,
                                  fill=0.0, base=0,
                                  pattern=[[0, 2], [1, 128]],
                                  channel_multiplier=-1)
                          # software pipeline (depth 2): PV for tile t-2 is
                          # emitted after scores(t), giving the exp->select
                          # chain ~1 extra tile of slack before the in-order
                          # PE reaches the dependent PV
                          pending.append((pt, qskip, t))
                          if len(pending) > 2:
                              emit_pv(*pending.pop(0))
                              if interleave:
                                  pop()
                      for args in pending:
                          emit_pv(*args)
                          if interleave:
                              pop()
                      # evict unnormalized PV; denominator to partition 32*pr
                      au = aup.tile([64, 1024], F32, tag=f"attu{pr}",
                                    name=f"attu_{rep}_{j}_{pr}")
                      nc.vector.tensor_copy(au[:], pv[0:64, :])
                      nc.vector.tensor_copy(denj[32 * pr:32 * pr + 1, :], pv[64:65, :])
                      attu.append(au)

                  if j == 3 and "2b" in op_gens:
                      queue.append(op_gens.pop("2b"))
                  # normalization: 1/den = exp(-ln(den)) on ACT, one op for all
                  # 4 pairs (rows 32*pr; other rows are garbage, never read).
                  # ACT is the right engine here: at j-ends the exps are done
                  # (ACT idle) while DVE still drains eviction/mul backlog.
                  lnd = dnp.tile([128, 1024], F32, tag="lnd", name=f"lnd_{rep}_{j}")
                  r_sb = dnp.tile([128, 1024], BF16, tag="rsb", name=f"r_{rep}_{j}")
                  nc.scalar.activation(lnd[0:97, :], denj[0:97, :], LN)
                  nc.scalar.activation(r_sb[0:97, :], lnd[0:97, :], EXP, scale=-1.0)
                  for pr in range(PAIRS):
                      for hh in range(2):
                          bc = psp.tile([64, 512], F32, tag="work", bufs=2,
                                        name=f"bc_{rep}_{j}_{pr}_{hh}")
                          nc.tensor.matmul(bc[:], ones128[32 * pr:32 * pr + 1, :],
                                           r_sb[32 * pr:32 * pr + 1,
                                                hh * 512:(hh + 1) * 512],
                                           start=True, stop=True,
                                           tile_position=(32 * pr, 0))
                          dst = ATT[64 * hh:64 * hh + 64,
                                    pr * T + j * 512:pr * T + (j + 1) * 512]
                          nc.vector.tensor_mul(dst, attu[pr][:, hh * 512:(hh + 1) * 512],
                                               bc[0:64, :])
                          if interleave:
                              pop()

                  if "d" in phases:
                      if j == 2:
                          # op(2) fills j3's last pair + the norm window;
                          # its evictions stay off ACT (exp-critical there)
                          op_gens["2a"] = outproj_gen(2, (0, 1), nb1_vec=True)
                          op_gens["2b"] = outproj_gen(2, (2, 3), nb1_vec=True)
                      elif j < NBLK - 1:
                          op_gens[j] = outproj_gen(j, nb1_vec=(j == 1))
                      else:
                          queue.append(outproj_gen(j))
                  if j == NBLK - 1:
                      for jj in sorted(op_gens, key=str):
                          queue.append(op_gens.pop(jj))
                      drain()

            for rep in range(reps):
                _emit_body(rep)

    # Exp and Ln are both in the natural_log_exp_and_others ACT table set,
    # but the table-load pass picks the first set containing each function
    # independently, thrashing tables between every exp batch and Ln.
    # Restrict Exp/Ln to the shared set (indexes preserved) for this build.
    import concourse.bacc as bacc_mod
    orig_tables = bacc_mod.get_activation_tables

    def _patched_tables(arch):
        tabs = orig_tables(arch)
        shared = tabs.get("natural_log_exp_and_others")
        if shared and EXP in shared and LN in shared:
            for name, s in tabs.items():
                if name != "natural_log_exp_and_others":
                    s.discard(EXP)
                    s.discard(LN)
        return tabs

    bacc_mod.get_activation_tables = _patched_tables
    try:
        nc.compile()
    finally:
        bacc_mod.get_activation_tables = orig_tables
    return nc


def _prep_inputs(x, freqs_cos, freqs_sin, wq, wk, wv, wo):
    """Host-side shard prep. Returns per-core input dicts."""
    import ml_dtypes
    BF = ml_dtypes.bfloat16
    x = np.asarray(x, dtype=np.float32)
    fc = np.asarray(freqs_cos, dtype=np.float32)
    fs = np.asarray(freqs_sin, dtype=np.float32)
    wq = np.asarray(wq, dtype=np.float32)
    wk = np.asarray(wk, dtype=np.float32)
    wv = np.asarray(wv, dtype=np.float32)
    wo = np.asarray(wo, dtype=np.float32)

    CSa = np.ascontiguousarray(np.tile(fc.T, (4, 1))).astype(BF)   # [128, S]
    SNa = np.ascontiguousarray(np.tile(fs.T, (4, 1))).astype(BF)
    xTb = [np.ascontiguousarray(x[b].T).astype(BF) for b in range(B)]

    in_maps = []
    for c in range(NCORES):
        b, g = divmod(c, NGRP)
        wcols = []
        for cg in range(2):
            h0 = HPG * g + 4 * cg       # first of 4 q heads in this cg
            kv = 2 * g + cg
            qcols = np.empty(256, dtype=np.int64)
            for h in range(4):
                for p in range(32):
                    qcols[32 * h + p] = (h0 + h) * HD + 2 * p            # evens
                    qcols[128 + 32 * h + p] = (h0 + h) * HD + 2 * p + 1  # odds
            kcols = np.empty(64, dtype=np.int64)
            kcols[:32] = HD * kv + 2 * np.arange(32)
            kcols[32:] = HD * kv + 2 * np.arange(32) + 1
            wcols.append(np.concatenate(
                [wq[:, qcols], wk[:, kcols], wv[:, HD * kv:HD * (kv + 1)]], axis=1))
        Wc = np.ascontiguousarray(np.concatenate(wcols, axis=1)).astype(BF)
        WOc = np.ascontiguousarray(wo[512 * g:512 * (g + 1), :]).astype(BF)
        in_maps.append({"xT": xTb[b], "W": Wc, "WO": WOc, "CS": CSa, "SN": SNa})
    return in_maps


def kernel(x, freqs_cos, freqs_sin, wq, wk, wv, wo):
    from concourse.bass_utils import run_bass_kernel_spmd

    if "nc" not in _CACHE:
        _CACHE["nc"] = _build()
    nc = _CACHE["nc"]
    in_maps = _prep_inputs(x, freqs_cos, freqs_sin, wq, wk, wv, wo)
    res = run_bass_kernel_spmd(nc, in_maps, list(range(NCORES)))
    out = np.zeros((B, S, D), dtype=np.float32)
    for c in range(NCORES):
        b = c // NGRP
        out[b] += np.asarray(res.results[c]["OUT"], dtype=np.float32)
    return out

